# revision 1
# baseline (speedup 1.0000x reference)
"""Trainium2 Bass kernel for nn_MAD_72679436582977 (retrieval_knn).

For each edge endpoint (src/tgt of 1024 edges) and each of 4 heads: find the
8 nearest neighbors (excluding self) among 50000 nodes in a 32-d embedding
space, logits (q - e_k).f_q, dist |q - e_k|, softmax(1 - dist) over
16 neighbors + 8 sentinels, sigmoid of head-mean weighted sum.

Strategy: data-parallel over edges across 8 cores (128 edges/core, SPMD, no
collectives). Per core:
  - approximate distance GEMM s~[q, n] = 2 q.e_n - |e_n|^2 in float32r
    (full PE rate; ~1e-4 relative error), K=33 (32 dims + norm row),
    supers of 2048 nodes in PSUM;
  - per-super top-8 via DVE max8 + max_index directly from PSUM
    -> 200 approximate candidates/row;
  - approximate top-16 of the candidates (covers the exact top-9 with large
    margin: rank-9 to rank-16 value gaps >> f32r error);
  - indirect-DMA gather of the 16 candidate rows [embed(32) | norm | pad]
    and EXACT fp32 recompute of s_k = 2 q.e_k - |e_k|^2 on DVE
    (q itself is the gathered rank-1 row = self);
  - exact top-9 among the 16, drop rank-1 (self) -> exact neighbor set;
  - epilogue on-device: dist = sqrt(qn - s), weights exp(1-dist), logits via
    gathered embeds . field, softmax-ratio with sentinel mass, head mean,
    sigmoid.

Host only shards edges, lays out inputs, and concatenates the 8 per-core
outputs.
"""
import os
import sys

sys.path.insert(0, "/opt/trn_rl_repo")

import numpy as np

import concourse.bass as bass
import concourse.bacc as bacc
import concourse.mybir as mybir
from concourse import tile
from concourse.bass import IndirectOffsetOnAxis

F32 = mybir.dt.float32
F32R = mybir.dt.float32r
U32 = mybir.dt.uint32

N_HEADS = 4
N_NODES = 50000
DIM = 32
N_BATCH = 1024
N_SENT = 8
N_CORES = 8

EDGES_PER_CORE = N_BATCH // N_CORES          # 128
SUPER = 2048                                  # PSUM super-chunk (4 banks)
N_SUPERS = 25
LAST_W = 1024                                 # last super is half-width
N_PAD = SUPER * (N_SUPERS - 1) + LAST_W       # 50176
M_TILES = N_HEADS * 2                         # (head, src/tgt) tiles of 128 rows
KC = DIM + 1                                  # contraction: 32 dims + norm row
N_CAND = N_SUPERS * 8                         # 200 approx candidates per row
EW = DIM + 2                                  # gathered row: embed|norm|pad

LAST = {}


def _build_program():
    nc = bacc.Bacc(None, num_swdge_queues=2)

    rhs_d = nc.dram_tensor("rhs_aug", [N_HEADS, KC, N_PAD], F32R,
                           kind="ExternalInput")
    embn_d = nc.dram_tensor("embn", [N_HEADS * N_NODES, EW], F32,
                            kind="ExternalInput")
    qpack_d = nc.dram_tensor("qpack", [M_TILES, KC, 128], F32R,
                             kind="ExternalInput")
    aux_d = nc.dram_tensor("aux", [M_TILES, 128, DIM + 2], F32,
                           kind="ExternalInput")

    preds_d = nc.dram_tensor("preds", [128, 1], F32, kind="ExternalOutput")
    dbg_gid_d = nc.dram_tensor("dbg_gid", [M_TILES, 128, 16], U32,
                               kind="ExternalOutput")
    dbg_s_d = nc.dram_tensor("dbg_s", [M_TILES, 128, 8], F32,
                             kind="ExternalOutput")

    with tile.TileContext(nc) as tc:
        with tc.tile_pool(name="const", bufs=1) as cpool, \
             tc.tile_pool(name="qp", bufs=3) as qpool, \
             tc.tile_pool(name="rhs", bufs=4) as rpool, \
             tc.tile_pool(name="cand", bufs=3) as candp, \
             tc.tile_pool(name="small", bufs=3) as sp, \
             tc.tile_pool(name="acc", bufs=1) as accp, \
             tc.tile_pool(name="ps", bufs=2, space="PSUM") as psp:

            # constants
            iota_cand = cpool.tile([128, N_CAND], U32, tag="iota_cand")
            nc.gpsimd.iota(iota_cand[:], pattern=[[1, N_CAND]], base=0,
                           channel_multiplier=0)
            iota16 = cpool.tile([128, 16], U32, tag="iota16")
            nc.gpsimd.iota(iota16[:], pattern=[[1, 16]], base=0,
                           channel_multiplier=0)
            neg_inf8 = cpool.tile([128, 8], F32, tag="neg_inf8")
            nc.vector.memset(neg_inf8[:], -1e30)

            # per-head gid offsets: gid = h*N_NODES + j*SUPER + within
            ioff_h = []
            for h in range(N_HEADS):
                t = cpool.tile([128, N_SUPERS, 8], F32, tag=f"ioff{h}")
                nc.gpsimd.iota(t[:], pattern=[[SUPER, N_SUPERS], [0, 8]],
                               base=h * N_NODES, channel_multiplier=0,
                               allow_small_or_imprecise_dtypes=True)
                ioff_h.append(t)

            numneg_all = accp.tile([128, M_TILES], F32, tag="numneg")
            wsum_all = accp.tile([128, M_TILES], F32, tag="wsum")

            for m in range(M_TILES):
                h = m // 2
                q_s = qpool.tile([KC, 128], F32R, tag="q")
                nc.sync.dma_start(out=q_s[:], in_=qpack_d[m])
                aux_s = sp.tile([128, DIM + 2], F32, tag="aux")
                nc.sync.dma_start(out=aux_s[:], in_=aux_d[m])
                qn_s = aux_s[:, 0:1]
                qf_s = aux_s[:, 1:2]
                f_s = aux_s[:, 2:DIM + 2]

                cand_v = candp.tile([128, N_CAND], F32, tag="cv")
                cand_i = candp.tile([128, N_CAND], U32, tag="ci")

                for j in range(N_SUPERS):
                    w = SUPER if j < N_SUPERS - 1 else LAST_W
                    rhs_s = rpool.tile([KC, w], F32R, tag="rhs")
                    nc.sync.dma_start(
                        out=rhs_s[:], in_=rhs_d[h, :, j * SUPER:j * SUPER + w])
                    psum = psp.tile([128, w], F32, tag="ps")
                    for b in range(w // 512):
                        sl = slice(b * 512, (b + 1) * 512)
                        nc.tensor.matmul(psum[:, sl], q_s[:], rhs_s[:, sl],
                                         start=True, stop=True)
                    csl = slice(j * 8, (j + 1) * 8)
                    nc.vector.max(cand_v[:, csl], psum[:])
                    nc.vector.max_index(cand_i[:, csl], cand_v[:, csl], psum[:])

                # global gids (fp32; < 2^24 so exact)
                cand_g = candp.tile([128, N_CAND], F32, tag="cg")
                nc.vector.tensor_tensor(
                    out=cand_g[:],
                    in0=cand_i[:].rearrange("p (a b) -> p a b", b=8),
                    in1=ioff_h[h][:],
                    op=mybir.AluOpType.add)

                # approx top-16 (rank 1 = self by construction), first-8
                # extraction/gathers fire before the second max8 round so the
                # gather chain starts as early as possible
                m16 = sp.tile([128, 16], F32, tag="m16")
                wpos16 = sp.tile([128, 16], U32, tag="wpos16")
                wgid_f = sp.tile([128, 16], F32, tag="wgidf")
                wgid = sp.tile([128, 16], U32, tag="wgid")
                scratch = candp.tile([128, N_CAND], F32, tag="scr")
                gath = sp.tile([128, 16, EW], F32, tag="gath")

                nc.vector.max(m16[:, 0:8], cand_v[:])
                nc.vector.max_index(wpos16[:, 0:8], m16[:, 0:8], cand_v[:])
                for k in range(8):
                    nc.vector.scalar_tensor_tensor(
                        out=scratch[:], in0=iota_cand[:],
                        scalar=wpos16[:, k:k + 1], in1=cand_g[:],
                        op0=mybir.AluOpType.is_equal,
                        op1=mybir.AluOpType.mult,
                        accum_out=wgid_f[:, k:k + 1])
                    nc.vector.tensor_copy(wgid[:, k:k + 1], wgid_f[:, k:k + 1])
                    nc.gpsimd.indirect_dma_start(
                        out=gath[:, k], out_offset=None,
                        in_=embn_d[:],
                        in_offset=IndirectOffsetOnAxis(ap=wgid[:, k:k + 1],
                                                       axis=0))
                vrep = candp.tile([128, N_CAND], F32, tag="vrep")
                nc.vector.match_replace(vrep[:], m16[:, 0:8], cand_v[:], -1e30)
                nc.vector.max(m16[:, 8:16], vrep[:])
                nc.vector.max_index(wpos16[:, 8:16], m16[:, 8:16], vrep[:])
                for k in range(8, 16):
                    nc.vector.scalar_tensor_tensor(
                        out=scratch[:], in0=iota_cand[:],
                        scalar=wpos16[:, k:k + 1], in1=cand_g[:],
                        op0=mybir.AluOpType.is_equal,
                        op1=mybir.AluOpType.mult,
                        accum_out=wgid_f[:, k:k + 1])
                    nc.vector.tensor_copy(wgid[:, k:k + 1], wgid_f[:, k:k + 1])
                    nc.gpsimd.indirect_dma_start(
                        out=gath[:, k], out_offset=None,
                        in_=embn_d[:],
                        in_offset=IndirectOffsetOnAxis(ap=wgid[:, k:k + 1],
                                                       axis=0))

                # exact s_k = 2 q.e_k - |e_k|^2 ; q = gathered rank-1 row
                prod16 = sp.tile([128, 16, DIM], F32, tag="prod16")
                nc.vector.tensor_tensor(
                    out=prod16[:], in0=gath[:, :, 0:DIM],
                    in1=gath[:, 0:1, 0:DIM].to_broadcast((128, 16, DIM)),
                    op=mybir.AluOpType.mult)
                dot16 = sp.tile([128, 16], F32, tag="dot16")
                nc.vector.tensor_reduce(dot16[:], prod16[:],
                                        axis=mybir.AxisListType.X,
                                        op=mybir.AluOpType.add)
                # mirror the reference's rounding: d2 = (qn + en) - 2*dot;
                # select on nd2 = 2*dot - (qn + en) == -d2 exactly.
                t16 = sp.tile([128, 16], F32, tag="t16")
                nc.vector.tensor_scalar(out=t16[:], in0=gath[:, :, DIM],
                                        scalar1=qn_s, scalar2=None,
                                        op0=mybir.AluOpType.add)
                s16 = sp.tile([128, 16], F32, tag="s16")
                nc.vector.scalar_tensor_tensor(
                    out=s16[:], in0=dot16[:], scalar=2.0,
                    in1=t16[:],
                    op0=mybir.AluOpType.mult, op1=mybir.AluOpType.subtract)

                # u_k = e_k . f_q for all 16
                prodf = sp.tile([128, 16, DIM], F32, tag="prodf")
                nc.vector.tensor_tensor(
                    out=prodf[:], in0=gath[:, :, 0:DIM],
                    in1=f_s.rearrange("p (o d) -> p o d", o=1).to_broadcast(
                        (128, 16, DIM)),
                    op=mybir.AluOpType.mult)
                u16 = sp.tile([128, 16], F32, tag="u16")
                nc.vector.tensor_reduce(u16[:], prodf[:],
                                        axis=mybir.AxisListType.X,
                                        op=mybir.AluOpType.add)

                # exact top-9, drop rank-1 (self). Winners then selected
                # by MARKING: match_replace flags the first occurrence of
                # each of the 8 w8 values (tie-exact, duplicate-safe, same
                # semantics as position extraction) -> sentinel mask.
                m1 = sp.tile([128, 1], F32, tag="m1")
                nc.vector.tensor_reduce(m1[:], s16[:], axis=mybir.AxisListType.X,
                                        op=mybir.AluOpType.max)
                m1x8 = sp.tile([128, 8], F32, tag="m1x8")
                nc.vector.tensor_copy(m1x8[:], neg_inf8[:])
                nc.vector.tensor_copy(m1x8[:, 0:1], m1[:])
                srep = sp.tile([128, 16], F32, tag="srep")
                nc.vector.match_replace(srep[:], m1x8[:], s16[:], -1e30)
                w8 = sp.tile([128, 8], F32, tag="w8")
                nc.vector.max(w8[:], srep[:])
                srep2 = sp.tile([128, 16], F32, tag="srep2")
                nc.vector.match_replace(srep2[:], w8[:], srep[:], 1e30)
                mask = sp.tile([128, 16], F32, tag="mask")
                nc.vector.tensor_scalar(out=mask[:], in0=srep2[:],
                                        scalar1=1e29, scalar2=None,
                                        op0=mybir.AluOpType.is_ge)

                # dist/weights over all 16, masked; clamp s16 <= 0 first
                # (self's exact s16 can round slightly positive -> sqrt NaN)
                s16c = sp.tile([128, 16], F32, tag="s16c")
                nc.vector.tensor_scalar(out=s16c[:], in0=s16[:], scalar1=0.0,
                                        scalar2=None, op0=mybir.AluOpType.min)
                dist16 = sp.tile([128, 16], F32, tag="dist16")
                nc.scalar.activation(dist16[:], s16c[:],
                                     mybir.ActivationFunctionType.Sqrt,
                                     bias=0.0, scale=-1.0)
                wexp16 = sp.tile([128, 16], F32, tag="wexp16")
                nc.scalar.activation(wexp16[:], dist16[:],
                                     mybir.ActivationFunctionType.Exp,
                                     bias=1.0, scale=-1.0)
                wm16 = sp.tile([128, 16], F32, tag="wm16")
                nc.vector.tensor_tensor(out=wm16[:], in0=wexp16[:], in1=mask[:],
                                        op=mybir.AluOpType.mult)
                scrap16 = sp.tile([128, 16], F32, tag="scrap16")
                nc.vector.scalar_tensor_tensor(
                    out=scrap16[:], in0=u16[:], scalar=qf_s, in1=wm16[:],
                    op0=mybir.AluOpType.subtract, op1=mybir.AluOpType.mult,
                    accum_out=numneg_all[:, m:m + 1])
                nc.vector.tensor_reduce(wsum_all[:, m:m + 1], wm16[:],
                                        axis=mybir.AxisListType.X,
                                        op=mybir.AluOpType.add)

                # debug: (gid+1)*mask so the test can recover the winner set
                gdbg = sp.tile([128, 16], F32, tag="gdbg")
                nc.vector.scalar_tensor_tensor(
                    out=gdbg[:], in0=wgid_f[:], scalar=1.0, in1=mask[:],
                    op0=mybir.AluOpType.add, op1=mybir.AluOpType.mult)
                gdbg_u = sp.tile([128, 16], U32, tag="gdbgu")
                nc.vector.tensor_copy(gdbg_u[:], gdbg[:])

                nc.sync.dma_start(out=dbg_gid_d[m], in_=gdbg_u[:])
                nc.sync.dma_start(out=dbg_s_d[m], in_=w8[:])

            # combine heads: pred = sigmoid(mean_h num_h / den_h)
            nsum2 = sp.tile([128, N_HEADS], F32, tag="nsum2")
            nc.vector.tensor_reduce(
                nsum2[:], numneg_all[:].rearrange("p (h e) -> p h e", e=2),
                axis=mybir.AxisListType.X, op=mybir.AluOpType.add)
            den = sp.tile([128, N_HEADS], F32, tag="den")
            nc.vector.tensor_reduce(
                den[:], wsum_all[:].rearrange("p (h e) -> p h e", e=2),
                axis=mybir.AxisListType.X, op=mybir.AluOpType.add)
            den8 = sp.tile([128, N_HEADS], F32, tag="den8")
            nc.vector.tensor_scalar(out=den8[:], in0=den[:],
                                    scalar1=float(N_SENT), scalar2=None,
                                    op0=mybir.AluOpType.add)
            rden = sp.tile([128, N_HEADS], F32, tag="rden")
            nc.vector.reciprocal(rden[:], den8[:])
            ratio = sp.tile([128, N_HEADS], F32, tag="ratio")
            nc.vector.tensor_tensor(out=ratio[:], in0=nsum2[:], in1=rden[:],
                                    op=mybir.AluOpType.mult)
            ssum = sp.tile([128, 1], F32, tag="ssum")
            nc.vector.tensor_reduce(ssum[:], ratio[:], axis=mybir.AxisListType.X,
                                    op=mybir.AluOpType.add)
            preds_s = sp.tile([128, 1], F32, tag="preds")
            nc.scalar.activation(preds_s[:], ssum[:],
                                 mybir.ActivationFunctionType.Sigmoid,
                                 bias=0.0, scale=-1.0 / N_HEADS)
            nc.sync.dma_start(out=preds_d[:], in_=preds_s[:])

    return nc


def _prep_inputs(embeds, field, edges):
    """Host-side layout prep + per-core sharding."""
    embeds = np.asarray(embeds, dtype=np.float32)
    field = np.asarray(field, dtype=np.float32)
    edges = np.asarray(edges)

    en = np.sum(np.square(embeds), axis=-1, dtype=np.float32)
    rhs_aug = np.empty((N_HEADS, KC, N_PAD), dtype=np.float32)
    rhs_aug[:, :DIM, :N_NODES] = embeds.transpose(0, 2, 1)
    rhs_aug[:, DIM, :N_NODES] = en
    rhs_aug[:, :DIM, N_NODES:] = 0.0
    rhs_aug[:, DIM, N_NODES:] = 1e9    # pad columns get s = -1e9

    embn = np.zeros((N_HEADS * N_NODES, EW), dtype=np.float32)
    embn[:, :DIM] = embeds.reshape(-1, DIM)
    embn[:, DIM] = en.reshape(-1)

    in_maps = []
    for c in range(N_CORES):
        sl = slice(c * EDGES_PER_CORE, (c + 1) * EDGES_PER_CORE)
        qpack = np.zeros((M_TILES, KC, 128), dtype=np.float32)
        aux = np.zeros((M_TILES, 128, DIM + 2), dtype=np.float32)
        for m in range(M_TILES):
            h, e = m // 2, m % 2
            nodes = edges[e, sl]
            q = embeds[h, nodes]                      # (128, 32)
            f = field[h, nodes]                       # (128, 32)
            qpack[m, :DIM] = (2.0 * q).T
            qpack[m, DIM] = -1.0
            aux[m, :, 0] = np.einsum('bd,bd->b', q, q)
            aux[m, :, 1] = np.einsum('bd,bd->b', q, f)
            aux[m, :, 2:] = f
        in_maps.append({
            "rhs_aug": rhs_aug, "embn": embn,
            "qpack": qpack, "aux": aux,
        })
    return in_maps


def kernel(embeds, field, edges):
    from concourse.bass_utils import run_bass_kernel_spmd

    nc = _build_program()
    nc.finalize()
    in_maps = _prep_inputs(embeds, field, edges)
    core_ids = list(range(N_CORES))
    trace = bool(os.environ.get("KNN_TRACE"))
    tmpdir = os.environ.get("KNN_TRACE_DIR") or None
    out = run_bass_kernel_spmd(nc, in_maps, core_ids, trace=trace,
                               tmpdir=tmpdir)
    LAST["results"] = out
    preds = np.concatenate(
        [out.results[c]["preds"][:, 0] for c in range(N_CORES)])
    return preds.astype(np.float32)



# revision 15
# speedup vs baseline: 1.1908x; 1.1908x over previous
"""Trainium2 Bass kernel for nn_MAD_72679436582977 (retrieval_knn).

For each edge endpoint (src/tgt of 1024 edges) and each of 4 heads: find the
8 nearest neighbors (excluding self) among 50000 nodes in a 32-d embedding
space, logits (q - e_k).f_q, dist |q - e_k|, softmax(1 - dist) over
16 neighbors + 8 sentinels, sigmoid of head-mean weighted sum.

Strategy: data-parallel over edges across 8 cores (128 edges/core, SPMD, no
collectives).  The distance GEMM s[q, n] = 2 q.e_n - |e_n|^2 runs in f32r
(full PE rate).  Candidate selection is a multi-engine reduction pipeline:

  - per 2048-node super: Act casts PSUM fp32 -> SBUF fp16; a 3-round
    halves max-fold (DVE / GpSimd, fp16 2x mode) pools groups of 8
    {f, f+256, ..., f+1792} down to 256 slots;
  - DVE max8 + max_index on the pooled 256 -> per-super top-8 group
    values + slots (dup-safe hardware semantics);
  - per m-tile: quantize values, pack (qv*8192 + globalslot) into fp32
    ints, top-16 via max8/match_replace/max8 (ties impossible: slot in
    low bits), decode slots -> 16 groups x 8 node gids;
  - ONE batched indirect DMA gathers all 128 candidate rows
    [embed(32) | norm | pad]; exact fp32 recompute s = 2 q.e - |e|^2
    (products on GpSimd, reduce on DVE); exact top-9, drop rank-1
    (self), winner gids via mask*(gid+1) max8;
  - second tiny gather of the 8 winners -> field dots, dist, weights,
    softmax-ratio with sentinel mass, head mean, sigmoid.

Containment of the true top-9 in the 16 gathered groups was validated
against the reference data (0/8192 failures, worst needed rank 14).
"""
import os
import sys

sys.path.insert(0, "/opt/trn_rl_repo")

import numpy as np

import concourse.bass as bass
import concourse.bacc as bacc
import concourse.mybir as mybir
from concourse import tile
from concourse.bass import IndirectOffsetOnAxis

F32 = mybir.dt.float32
F32R = mybir.dt.float32r
F16 = mybir.dt.float16
U32 = mybir.dt.uint32

N_HEADS = 4
N_NODES = 50000
DIM = 32
N_BATCH = 1024
N_SENT = 8
N_CORES = 8

EDGES_PER_CORE = N_BATCH // N_CORES          # 128
SUP = 2048                                    # super width (4 PSUM banks)
N_SUPERS = 25
N_PAD = SUP * N_SUPERS                        # 51200
M_TILES = N_HEADS * 2                         # (head, src/tgt) tiles
KC = DIM + 1                                  # contraction: 32 dims + norm row
N_CAND = N_SUPERS * 8                         # 200 group candidates per row
N_WIN = 16                                    # winner groups kept
N_GATH = N_WIN * 8                            # 128 gathered candidate rows
EW = DIM + 2                                  # gathered row: embed|norm|pad
AUXW = 2 + DIM + DIM                          # qn | qf | f | q

TWO23 = 12582912.0   # 1.5*2^23: round-to-int magic (ulp=1 across the sum)

# per-super fold engine assignment: 'G' = all folds on GpSimd,
# 'M' = fold1 DVE + folds2,3 GpSimd, 'D' = all folds on DVE (from PSUM,
# no Act cast).  Tuned for Act/DVE/GpSimd balance.
SUPER_TYPE = list("MMMMM MMMMM MMMMM MMMMM MMMMM".replace(" ", ""))
assert len(SUPER_TYPE) == N_SUPERS

LAST = {}


def _build_program(debug=False):
    nc = bacc.Bacc(None, num_swdge_queues=2)

    rhs_d = nc.dram_tensor("rhs_aug", [N_HEADS, KC, N_PAD], F32R,
                           kind="ExternalInput")
    embn_d = nc.dram_tensor("embn", [N_HEADS * N_PAD, EW], F32,
                            kind="ExternalInput")
    qpack_d = nc.dram_tensor("qpack", [M_TILES, KC, 128], F32R,
                             kind="ExternalInput")
    aux_d = nc.dram_tensor("aux", [M_TILES, 128, AUXW], F32,
                           kind="ExternalInput")

    preds_d = nc.dram_tensor("preds", [128, 1], F32, kind="ExternalOutput")
    dbg_gid_d = nc.dram_tensor("dbg_gid", [M_TILES, 128, 8], U32,
                               kind="ExternalOutput")
    dbg_s_d = nc.dram_tensor("dbg_s", [M_TILES, 128, 8], F32,
                             kind="ExternalOutput")
    if debug:
        dbg_cv_d = nc.dram_tensor("dbg_cv", [128, N_CAND], F32,
                                  kind="ExternalOutput")
        dbg_cs_d = nc.dram_tensor("dbg_cs", [128, N_CAND], U32,
                                  kind="ExternalOutput")
        dbg_pk_d = nc.dram_tensor("dbg_pk", [128, N_CAND], F32,
                                  kind="ExternalOutput")
        dbg_w16_d = nc.dram_tensor("dbg_w16", [128, N_WIN], F32,
                                   kind="ExternalOutput")
        dbg_gidu_d = nc.dram_tensor("dbg_gidu", [128, N_GATH], U32,
                                    kind="ExternalOutput")
        dbg_s128_d = nc.dram_tensor("dbg_s128", [128, N_GATH], F32,
                                    kind="ExternalOutput")
        dbg_pld_d = nc.dram_tensor("dbg_pld", [128, 256], F32,
                                   kind="ExternalOutput")

    with tile.TileContext(nc) as tc:
        with tc.tile_pool(name="const", bufs=1) as cpool, \
             tc.tile_pool(name="qp", bufs=1) as qpool, \
             tc.tile_pool(name="rhs", bufs=3) as rpool, \
             tc.tile_pool(name="hc", bufs=3) as hpool, \
             tc.tile_pool(name="fold", bufs=2) as fpool, \
             tc.tile_pool(name="cand", bufs=2) as candp, \
             tc.tile_pool(name="fin", bufs=2) as finp, \
             tc.tile_pool(name="gath", bufs=2) as gpool, \
             tc.tile_pool(name="prod", bufs=2) as ppool, \
             tc.tile_pool(name="acc", bufs=1) as accp, \
             tc.tile_pool(name="ps", bufs=2, space="PSUM") as psp:

            # ---- constants ----
            # per-head fold-group member offsets {0,256,...,1792} + h*N_PAD
            ioff_h = []
            for h in range(N_HEADS):
                t = cpool.tile([128, 8], F32, tag=f"ioff{h}")
                nc.gpsimd.iota(t[:], pattern=[[256, 8]], base=h * N_PAD,
                               channel_multiplier=0,
                               allow_small_or_imprecise_dtypes=True)
                ioff_h.append(t)
            # j*256 offset per candidate position (200-array)
            joff = cpool.tile([128, N_SUPERS, 8], F32, tag="joff")
            nc.gpsimd.iota(joff[:], pattern=[[256, N_SUPERS], [0, 8]], base=0,
                           channel_multiplier=0,
                           allow_small_or_imprecise_dtypes=True)
            neg_inf8 = cpool.tile([128, 8], F32, tag="neg_inf8")
            nc.vector.memset(neg_inf8[:], -1e30)

            # ---- query tiles (all m upfront; small) ----
            q_s = []
            aux_s = []
            for m in range(M_TILES):
                qt = qpool.tile([KC, 128], F32R, tag=f"q{m}")
                nc.sync.dma_start(out=qt[:], in_=qpack_d[m])
                q_s.append(qt)
                at = qpool.tile([128, AUXW], F32, tag=f"aux{m}")
                nc.sync.dma_start(out=at[:], in_=aux_d[m])
                aux_s.append(at)

            numneg_all = accp.tile([128, M_TILES], F32, tag="numneg")
            wsum_all = accp.tile([128, M_TILES], F32, tag="wsum")

            for h in range(N_HEADS):
                # per-m-tile candidate stores for both endpoints
                cv = []
                cs = []
                for e in range(2):
                    cvt = candp.tile([128, N_SUPERS, 8], F16, tag=f"cv{e}")
                    cv.append(cvt)
                    cst = candp.tile([128, N_SUPERS, 8], U32, tag=f"cs{e}")
                    cs.append(cst)

                for j in range(N_SUPERS):
                    rhs_s = rpool.tile([KC, SUP], F32R, tag="rhs")
                    nc.sync.dma_start(
                        out=rhs_s[:], in_=rhs_d[h, :, j * SUP:(j + 1) * SUP])
                    for e in range(2):
                        m = 2 * h + e
                        psum = psp.tile([128, SUP], F32, tag="ps")
                        for b in range(SUP // 512):
                            sl = slice(b * 512, (b + 1) * 512)
                            nc.tensor.matmul(psum[:, sl], q_s[m][:], rhs_s[:, sl],
                                             start=True, stop=True)

                        ty = SUPER_TYPE[j]
                        f1 = fpool.tile([128, 1024], F16, tag="f1")
                        f2 = fpool.tile([128, 512], F16, tag="f2")
                        pld = fpool.tile([128, 256], F16, tag="pld")
                        if ty in ("G", "M"):
                            hcv = hpool.tile([128, SUP], F16, tag="hc")
                            nc.scalar.activation(
                                hcv[:], psum[:],
                                mybir.ActivationFunctionType.Copy,
                                bias=0.0, scale=1.0)
                            nc.vector.tensor_tensor(
                                out=f1[:], in0=hcv[:, 0:1024],
                                in1=hcv[:, 1024:2048],
                                op=mybir.AluOpType.max)
                        else:  # 'D': DVE-only (PSUM allows one input per op)
                            c1 = fpool.tile([128, 1024], F16, tag="c1")
                            nc.vector.tensor_copy(c1[:], psum[:, 0:1024])
                            nc.vector.tensor_tensor(
                                out=f1[:], in0=c1[:],
                                in1=psum[:, 1024:2048],
                                op=mybir.AluOpType.max)
                        nc.vector.tensor_tensor(
                            out=f2[:], in0=f1[:, 0:512], in1=f1[:, 512:1024],
                            op=mybir.AluOpType.max)
                        nc.vector.tensor_tensor(
                            out=pld[:], in0=f2[:, 0:256], in1=f2[:, 256:512],
                            op=mybir.AluOpType.max)

                        nc.vector.max(cv[e][:, j], pld[:])
                        nc.vector.max_index(cs[e][:, j], cv[e][:, j], pld[:])
                        if debug and h == 0 and j == 0 and e == 0:
                            pldf = fpool.tile([128, 256], F32, tag="pldf")
                            nc.vector.tensor_copy(pldf[:], pld[:])
                            nc.sync.dma_start(out=dbg_pld_d[:], in_=pldf[:])

                for e in range(2):
                    m = 2 * h + e
                    qn_s = aux_s[m][:, 0:1]
                    qf_s = aux_s[m][:, 1:2]
                    f_row = aux_s[m][:, 2:2 + DIM]
                    q_row = aux_s[m][:, 2 + DIM:2 + 2 * DIM]

                    # ---- pack candidates: qv*8192 + gslot ----
                    cvf = finp.tile([128, N_CAND], F32, tag="cvf")
                    # qv = round(clip((v+42)*32, 0, 2047))
                    nc.scalar.activation(
                        cvf[:], cv[e][:].rearrange("p a b -> p (a b)"),
                        mybir.ActivationFunctionType.Copy,
                        bias=0.0, scale=32.0)
                    nc.vector.tensor_scalar(
                        out=cvf[:], in0=cvf[:], scalar1=1344.0,
                        scalar2=TWO23, op0=mybir.AluOpType.add,
                        op1=mybir.AluOpType.add)
                    nc.vector.tensor_scalar(
                        out=cvf[:], in0=cvf[:], scalar1=TWO23,
                        scalar2=None, op0=mybir.AluOpType.subtract)
                    nc.vector.tensor_scalar(
                        out=cvf[:], in0=cvf[:], scalar1=2047.0,
                        scalar2=0.0, op0=mybir.AluOpType.min,
                        op1=mybir.AluOpType.max)
                    slotf = finp.tile([128, N_CAND], F32, tag="slotf")
                    nc.vector.tensor_copy(
                        slotf[:], cs[e][:].rearrange("p a b -> p (a b)"))
                    gslot = finp.tile([128, N_CAND], F32, tag="gslot")
                    nc.vector.tensor_tensor(
                        out=gslot[:], in0=slotf[:],
                        in1=joff[:].rearrange("p a b -> p (a b)"),
                        op=mybir.AluOpType.add)
                    packed = finp.tile([128, N_CAND], F32, tag="packed")
                    nc.vector.scalar_tensor_tensor(
                        out=packed[:], in0=cvf[:], scalar=8192.0,
                        in1=gslot[:], op0=mybir.AluOpType.mult,
                        op1=mybir.AluOpType.add)
                    if debug and m == 0:
                        cvdbg = finp.tile([128, N_CAND], F32, tag="cvdbg")
                        nc.vector.tensor_copy(
                            cvdbg[:], cv[e][:].rearrange("p a b -> p (a b)"))
                        nc.sync.dma_start(out=dbg_cv_d[:], in_=cvdbg[:])
                        csdbg = finp.tile([128, N_CAND], U32, tag="csdbg")
                        nc.vector.tensor_copy(
                            csdbg[:], cs[e][:].rearrange("p a b -> p (a b)"))
                        nc.sync.dma_start(out=dbg_cs_d[:], in_=csdbg[:])
                        nc.sync.dma_start(out=dbg_pk_d[:], in_=packed[:])

                    # ---- top-16 packed (ties impossible) ----
                    w16 = finp.tile([128, N_WIN], F32, tag="w16")
                    nc.vector.max(w16[:, 0:8], packed[:])
                    prep = finp.tile([128, N_CAND], F32, tag="prep")
                    nc.vector.match_replace(prep[:], w16[:, 0:8], packed[:],
                                            -1e30)
                    nc.vector.max(w16[:, 8:16], prep[:])

                    # ---- decode: gslot16 = w16 mod 8192 -> (j, f) -> base ----
                    qv16 = finp.tile([128, N_WIN], F32, tag="qv16")
                    nc.scalar.activation(qv16[:], w16[:],
                                         mybir.ActivationFunctionType.Copy,
                                         bias=0.0, scale=1.0 / 8192.0)
                    nc.vector.tensor_scalar(
                        out=qv16[:], in0=qv16[:], scalar1=-0.499969482421875,
                        scalar2=TWO23, op0=mybir.AluOpType.add,
                        op1=mybir.AluOpType.add)
                    nc.vector.tensor_scalar(
                        out=qv16[:], in0=qv16[:], scalar1=TWO23,
                        scalar2=None, op0=mybir.AluOpType.subtract)
                    g16 = finp.tile([128, N_WIN], F32, tag="g16")
                    nc.vector.scalar_tensor_tensor(
                        out=g16[:], in0=qv16[:], scalar=-8192.0,
                        in1=w16[:], op0=mybir.AluOpType.mult,
                        op1=mybir.AluOpType.add)
                    # j16 = floor(g16/256); f16 = g16 - 256*j16
                    j16 = finp.tile([128, N_WIN], F32, tag="j16")
                    nc.scalar.activation(j16[:], g16[:],
                                         mybir.ActivationFunctionType.Copy,
                                         bias=0.0, scale=1.0 / 256.0)
                    nc.vector.tensor_scalar(
                        out=j16[:], in0=j16[:], scalar1=-0.498046875,
                        scalar2=TWO23, op0=mybir.AluOpType.add,
                        op1=mybir.AluOpType.add)
                    nc.vector.tensor_scalar(
                        out=j16[:], in0=j16[:], scalar1=TWO23,
                        scalar2=None, op0=mybir.AluOpType.subtract)
                    # base16 = g16 + 1792*j16  (= j*2048 + f)
                    base16 = finp.tile([128, N_WIN], F32, tag="base16")
                    nc.vector.scalar_tensor_tensor(
                        out=base16[:], in0=j16[:], scalar=1792.0,
                        in1=g16[:], op0=mybir.AluOpType.mult,
                        op1=mybir.AluOpType.add)
                    # gid128 = base16 + {0,256,...,1792} + h*N_PAD
                    gidf = finp.tile([128, N_WIN, 8], F32, tag="gidf")
                    nc.vector.tensor_tensor(
                        out=gidf[:],
                        in0=base16[:].rearrange("p (a b) -> p a b", b=1)
                            .to_broadcast((128, N_WIN, 8)),
                        in1=ioff_h[h][:].rearrange("p (a b) -> p a b", a=1)
                            .to_broadcast((128, N_WIN, 8)),
                        op=mybir.AluOpType.add)
                    gidu = finp.tile([128, N_GATH], U32, tag="gidu")
                    nc.vector.tensor_copy(
                        gidu[:], gidf[:].rearrange("p a b -> p (a b)"))
                    if debug and m == 0:
                        nc.sync.dma_start(out=dbg_w16_d[:], in_=w16[:])
                        nc.sync.dma_start(out=dbg_gidu_d[:], in_=gidu[:])

                    # ---- batched gather of all 128 candidate rows ----
                    gath = gpool.tile([128, N_GATH, EW], F32, tag="gath")
                    nc.gpsimd.indirect_dma_start(
                        out=gath[:], out_offset=None,
                        in_=embn_d[:],
                        in_offset=IndirectOffsetOnAxis(ap=gidu[:], axis=0))

                    # ---- exact recompute s = 2 q.e - (qn + en) ----
                    prod = ppool.tile([128, N_GATH, DIM], F32, tag="prod")
                    nc.vector.tensor_tensor(
                        out=prod[:], in0=gath[:, :, 0:DIM],
                        in1=q_row.rearrange("p (o d) -> p o d", o=1)
                            .to_broadcast((128, N_GATH, DIM)),
                        op=mybir.AluOpType.mult)
                    dot = finp.tile([128, N_GATH], F32, tag="dot")
                    nc.vector.tensor_reduce(dot[:], prod[:],
                                            axis=mybir.AxisListType.X,
                                            op=mybir.AluOpType.add)
                    t128 = finp.tile([128, N_GATH], F32, tag="t128")
                    nc.vector.tensor_scalar(out=t128[:], in0=gath[:, :, DIM],
                                            scalar1=qn_s, scalar2=None,
                                            op0=mybir.AluOpType.add)
                    s128 = finp.tile([128, N_GATH], F32, tag="s128")
                    nc.vector.scalar_tensor_tensor(
                        out=s128[:], in0=dot[:], scalar=2.0, in1=t128[:],
                        op0=mybir.AluOpType.mult,
                        op1=mybir.AluOpType.subtract)
                    if debug and m == 0:
                        nc.sync.dma_start(out=dbg_s128_d[:], in_=s128[:])

                    # ---- exact top-9, drop rank-1 (self) ----
                    m1 = finp.tile([128, 1], F32, tag="m1")
                    nc.vector.tensor_reduce(m1[:], s128[:],
                                            axis=mybir.AxisListType.X,
                                            op=mybir.AluOpType.max)
                    m1x8 = finp.tile([128, 8], F32, tag="m1x8")
                    nc.vector.tensor_copy(m1x8[:], neg_inf8[:])
                    nc.vector.tensor_copy(m1x8[:, 0:1], m1[:])
                    srep = finp.tile([128, N_GATH], F32, tag="srep")
                    nc.vector.match_replace(srep[:], m1x8[:], s128[:], -1e30)
                    w8 = finp.tile([128, 8], F32, tag="w8")
                    nc.vector.max(w8[:], srep[:])
                    srep2 = finp.tile([128, N_GATH], F32, tag="srep2")
                    nc.vector.match_replace(srep2[:], w8[:], srep[:], 1e30)
                    mask = finp.tile([128, N_GATH], F32, tag="mask")
                    nc.vector.tensor_scalar(out=mask[:], in0=srep2[:],
                                            scalar1=1e29, scalar2=None,
                                            op0=mybir.AluOpType.is_ge)
                    gidsel = finp.tile([128, N_GATH], F32, tag="gidsel")
                    nc.vector.scalar_tensor_tensor(
                        out=gidsel[:], in0=gidf[:].rearrange("p a b -> p (a b)"),
                        scalar=1.0, in1=mask[:],
                        op0=mybir.AluOpType.add, op1=mybir.AluOpType.mult)
                    wgidf = finp.tile([128, 8], F32, tag="wgidf")
                    nc.vector.max(wgidf[:], gidsel[:])
                    wgidu = finp.tile([128, 8], U32, tag="wgidu")
                    nc.vector.tensor_scalar(
                        out=wgidu[:], in0=wgidf[:], scalar1=-1.0,
                        scalar2=None, op0=mybir.AluOpType.add)
                    nc.sync.dma_start(out=dbg_gid_d[m], in_=wgidu[:])
                    nc.sync.dma_start(out=dbg_s_d[m], in_=w8[:])

                    # ---- gather the 8 winners, field dots, weights ----
                    g2 = gpool.tile([128, 8, EW], F32, tag="g2")
                    nc.gpsimd.indirect_dma_start(
                        out=g2[:], out_offset=None,
                        in_=embn_d[:],
                        in_offset=IndirectOffsetOnAxis(ap=wgidu[:], axis=0))
                    prod8 = finp.tile([128, 8, DIM], F32, tag="prod8")
                    nc.vector.tensor_tensor(
                        out=prod8[:], in0=g2[:, :, 0:DIM],
                        in1=q_row.rearrange("p (o d) -> p o d", o=1)
                            .to_broadcast((128, 8, DIM)),
                        op=mybir.AluOpType.mult)
                    dot8 = finp.tile([128, 8], F32, tag="dot8")
                    nc.vector.tensor_reduce(dot8[:], prod8[:],
                                            axis=mybir.AxisListType.X,
                                            op=mybir.AluOpType.add)
                    t8 = finp.tile([128, 8], F32, tag="t8")
                    nc.vector.tensor_scalar(out=t8[:], in0=g2[:, :, DIM],
                                            scalar1=qn_s, scalar2=None,
                                            op0=mybir.AluOpType.add)
                    s8 = finp.tile([128, 8], F32, tag="s8")
                    nc.vector.scalar_tensor_tensor(
                        out=s8[:], in0=dot8[:], scalar=2.0, in1=t8[:],
                        op0=mybir.AluOpType.mult,
                        op1=mybir.AluOpType.subtract)
                    nc.vector.tensor_scalar(out=s8[:], in0=s8[:], scalar1=0.0,
                                            scalar2=None,
                                            op0=mybir.AluOpType.min)
                    dist8 = finp.tile([128, 8], F32, tag="dist8")
                    nc.scalar.activation(dist8[:], s8[:],
                                         mybir.ActivationFunctionType.Sqrt,
                                         bias=0.0, scale=-1.0)
                    wexp8 = finp.tile([128, 8], F32, tag="wexp8")
                    nc.scalar.activation(wexp8[:], dist8[:],
                                         mybir.ActivationFunctionType.Exp,
                                         bias=1.0, scale=-1.0)
                    prodf8 = finp.tile([128, 8, DIM], F32, tag="prodf8")
                    nc.vector.tensor_tensor(
                        out=prodf8[:], in0=g2[:, :, 0:DIM],
                        in1=f_row.rearrange("p (o d) -> p o d", o=1)
                            .to_broadcast((128, 8, DIM)),
                        op=mybir.AluOpType.mult)
                    u8 = finp.tile([128, 8], F32, tag="u8")
                    nc.vector.tensor_reduce(u8[:], prodf8[:],
                                            axis=mybir.AxisListType.X,
                                            op=mybir.AluOpType.add)
                    scrap8 = finp.tile([128, 8], F32, tag="scrap8")
                    nc.vector.scalar_tensor_tensor(
                        out=scrap8[:], in0=u8[:], scalar=qf_s, in1=wexp8[:],
                        op0=mybir.AluOpType.subtract,
                        op1=mybir.AluOpType.mult,
                        accum_out=numneg_all[:, m:m + 1])
                    nc.vector.tensor_reduce(wsum_all[:, m:m + 1], wexp8[:],
                                            axis=mybir.AxisListType.X,
                                            op=mybir.AluOpType.add)

            # ---- combine heads: pred = sigmoid(mean_h num_h / den_h) ----
            sp = finp
            nsum2 = sp.tile([128, N_HEADS], F32, tag="nsum2")
            nc.vector.tensor_reduce(
                nsum2[:], numneg_all[:].rearrange("p (h e) -> p h e", e=2),
                axis=mybir.AxisListType.X, op=mybir.AluOpType.add)
            den = sp.tile([128, N_HEADS], F32, tag="den")
            nc.vector.tensor_reduce(
                den[:], wsum_all[:].rearrange("p (h e) -> p h e", e=2),
                axis=mybir.AxisListType.X, op=mybir.AluOpType.add)
            den8 = sp.tile([128, N_HEADS], F32, tag="den8")
            nc.vector.tensor_scalar(out=den8[:], in0=den[:],
                                    scalar1=float(N_SENT), scalar2=None,
                                    op0=mybir.AluOpType.add)
            rden = sp.tile([128, N_HEADS], F32, tag="rden")
            nc.vector.reciprocal(rden[:], den8[:])
            ratio = sp.tile([128, N_HEADS], F32, tag="ratio")
            nc.vector.tensor_tensor(out=ratio[:], in0=nsum2[:], in1=rden[:],
                                    op=mybir.AluOpType.mult)
            ssum = sp.tile([128, 1], F32, tag="ssum")
            nc.vector.tensor_reduce(ssum[:], ratio[:],
                                    axis=mybir.AxisListType.X,
                                    op=mybir.AluOpType.add)
            preds_s = sp.tile([128, 1], F32, tag="preds")
            nc.scalar.activation(preds_s[:], ssum[:],
                                 mybir.ActivationFunctionType.Sigmoid,
                                 bias=0.0, scale=-1.0 / N_HEADS)
            nc.sync.dma_start(out=preds_d[:], in_=preds_s[:])

    return nc


def _prep_inputs(embeds, field, edges):
    """Host-side layout prep + per-core sharding."""
    embeds = np.asarray(embeds, dtype=np.float32)
    field = np.asarray(field, dtype=np.float32)
    edges = np.asarray(edges)

    en = np.sum(np.square(embeds), axis=-1, dtype=np.float32)
    rhs_aug = np.empty((N_HEADS, KC, N_PAD), dtype=np.float32)
    rhs_aug[:, :DIM, :N_NODES] = embeds.transpose(0, 2, 1)
    rhs_aug[:, DIM, :N_NODES] = en
    rhs_aug[:, :DIM, N_NODES:] = 0.0
    rhs_aug[:, DIM, N_NODES:] = 60000.0  # pad columns: s = -60000 (fp16-safe)

    embn = np.zeros((N_HEADS * N_PAD, EW), dtype=np.float32)
    embn3 = embn.reshape(N_HEADS, N_PAD, EW)
    embn3[:, :N_NODES, :DIM] = embeds
    embn3[:, :N_NODES, DIM] = en
    embn3[:, N_NODES:, DIM] = 60000.0

    in_maps = []
    for c in range(N_CORES):
        sl = slice(c * EDGES_PER_CORE, (c + 1) * EDGES_PER_CORE)
        qpack = np.zeros((M_TILES, KC, 128), dtype=np.float32)
        aux = np.zeros((M_TILES, 128, AUXW), dtype=np.float32)
        for m in range(M_TILES):
            h, e = m // 2, m % 2
            nodes = edges[e, sl]
            q = embeds[h, nodes]                      # (128, 32)
            f = field[h, nodes]                       # (128, 32)
            qpack[m, :DIM] = (2.0 * q).T
            qpack[m, DIM] = -1.0
            aux[m, :, 0] = np.einsum('bd,bd->b', q, q)
            aux[m, :, 1] = np.einsum('bd,bd->b', q, f)
            aux[m, :, 2:2 + DIM] = f
            aux[m, :, 2 + DIM:] = q
        in_maps.append({
            "rhs_aug": rhs_aug, "embn": embn,
            "qpack": qpack, "aux": aux,
        })
    return in_maps


def kernel(embeds, field, edges):
    from concourse.bass_utils import run_bass_kernel_spmd

    nc = _build_program()
    nc.finalize()
    in_maps = _prep_inputs(embeds, field, edges)
    core_ids = list(range(N_CORES))
    trace = bool(os.environ.get("KNN_TRACE"))
    tmpdir = os.environ.get("KNN_TRACE_DIR") or None
    out = run_bass_kernel_spmd(nc, in_maps, core_ids, trace=trace,
                               tmpdir=tmpdir)
    LAST["results"] = out
    preds = np.concatenate(
        [out.results[c]["preds"][:, 0] for c in range(N_CORES)])
    return preds.astype(np.float32)


# revision 21
# speedup vs baseline: 1.3360x; 1.1219x over previous
"""Trainium2 Bass kernel for nn_MAD_72679436582977 (retrieval_knn).

For each edge endpoint (src/tgt of 1024 edges) and each of 4 heads: find the
8 nearest neighbors (excluding self) among 50000 nodes in a 32-d embedding
space, logits (q - e_k).f_q, dist |q - e_k|, softmax(1 - dist) over
16 neighbors + 8 sentinels, sigmoid of head-mean weighted sum.

Strategy: data-parallel over edges across 8 cores (128 edges/core, SPMD, no
collectives).  The distance GEMM s[q, n] = 2 q.e_n - |e_n|^2 runs in f32r
(full PE rate).  Candidate selection is a multi-engine reduction pipeline:

  - per 2048-node super: Act casts PSUM fp32 -> SBUF fp16; a 3-round
    halves max-fold (DVE / GpSimd, fp16 2x mode) pools groups of 8
    {f, f+256, ..., f+1792} down to 256 slots;
  - DVE max8 + max_index on the pooled 256 -> per-super top-8 group
    values + slots (dup-safe hardware semantics);
  - per m-tile: quantize values, pack (qv*8192 + globalslot) into fp32
    ints, top-16 via max8/match_replace/max8 (ties impossible: slot in
    low bits), decode slots -> 16 groups x 8 node gids;
  - ONE batched indirect DMA gathers all 128 candidate rows
    [embed(32) | norm | pad]; exact fp32 recompute s = 2 q.e - |e|^2
    (products on GpSimd, reduce on DVE); exact top-9, drop rank-1
    (self), winner gids via mask*(gid+1) max8;
  - second tiny gather of the 8 winners -> field dots, dist, weights,
    softmax-ratio with sentinel mass, head mean, sigmoid.

Containment of the true top-9 in the 16 gathered groups was validated
against the reference data (0/8192 failures, worst needed rank 14).
"""
import os
import sys

sys.path.insert(0, "/opt/trn_rl_repo")

import numpy as np

import concourse.bass as bass
import concourse.bacc as bacc
import concourse.mybir as mybir
from concourse import tile
from concourse.bass import IndirectOffsetOnAxis

F32 = mybir.dt.float32
F32R = mybir.dt.float32r
F16 = mybir.dt.float16
U32 = mybir.dt.uint32

N_HEADS = 4
N_NODES = 50000
DIM = 32
N_BATCH = 1024
N_SENT = 8
N_CORES = 8

EDGES_PER_CORE = N_BATCH // N_CORES          # 128
SUP = 2048                                    # super width (4 PSUM banks)
N_SUPERS = 25
N_PAD = SUP * N_SUPERS                        # 51200
M_TILES = N_HEADS * 2                         # (head, src/tgt) tiles
KC = DIM + 1                                  # contraction: 32 dims + norm row
N_CAND = N_SUPERS * 8                         # 200 group candidates per row
N_WIN = 16                                    # winner groups kept
N_GATH = N_WIN * 8                            # 128 gathered candidate rows
EW = DIM + 2                                  # gathered row: embed|norm|pad
AUXW = 2 + DIM + DIM                          # qn | qf | f | q

TWO23 = 12582912.0   # 1.5*2^23: round-to-int magic (ulp=1 across the sum)

# per-super fold engine assignment: 'G' = all folds on GpSimd,
# 'M' = fold1 DVE + folds2,3 GpSimd, 'D' = all folds on DVE (from PSUM,
# no Act cast).  Tuned for Act/DVE/GpSimd balance.
SUPER_TYPE = list("MMMMM MMMMM MMMMM MMMMM MMMMM".replace(" ", ""))
assert len(SUPER_TYPE) == N_SUPERS

LAST = {}


def _build_program(debug=False):
    nc = bacc.Bacc(None, num_swdge_queues=2)

    rhs_d = nc.dram_tensor("rhs_aug", [N_HEADS, KC, N_PAD], F32R,
                           kind="ExternalInput")
    embn_d = nc.dram_tensor("embn", [N_HEADS * N_PAD, EW], F32,
                            kind="ExternalInput")
    # group-major table: row (h, j, f) = the 8 group members' [emb|en|pad]
    embg_d = nc.dram_tensor("embg", [N_HEADS * N_SUPERS * 256, 8 * EW], F32,
                            kind="ExternalInput")
    qpack_d = nc.dram_tensor("qpack", [M_TILES, KC, 128], F32R,
                             kind="ExternalInput")
    aux_d = nc.dram_tensor("aux", [M_TILES, 128, AUXW], F32,
                           kind="ExternalInput")

    preds_d = nc.dram_tensor("preds", [128, 1], F32, kind="ExternalOutput")
    dbg_gid_d = nc.dram_tensor("dbg_gid", [M_TILES, 128, 8], U32,
                               kind="ExternalOutput")
    dbg_s_d = nc.dram_tensor("dbg_s", [M_TILES, 128, 8], F32,
                             kind="ExternalOutput")
    if debug:
        dbg_cv_d = nc.dram_tensor("dbg_cv", [128, N_CAND], F32,
                                  kind="ExternalOutput")
        dbg_cs_d = nc.dram_tensor("dbg_cs", [128, N_CAND], U32,
                                  kind="ExternalOutput")
        dbg_pk_d = nc.dram_tensor("dbg_pk", [128, N_CAND], F32,
                                  kind="ExternalOutput")
        dbg_w16_d = nc.dram_tensor("dbg_w16", [128, N_WIN], F32,
                                   kind="ExternalOutput")
        dbg_gidu_d = nc.dram_tensor("dbg_gidu", [128, N_GATH], U32,
                                    kind="ExternalOutput")
        dbg_s128_d = nc.dram_tensor("dbg_s128", [128, N_GATH], F32,
                                    kind="ExternalOutput")
        dbg_pld_d = nc.dram_tensor("dbg_pld", [128, 256], F32,
                                   kind="ExternalOutput")

    with tile.TileContext(nc) as tc:
        with tc.tile_pool(name="const", bufs=1) as cpool, \
             tc.tile_pool(name="qp", bufs=1) as qpool, \
             tc.tile_pool(name="rhs", bufs=3) as rpool, \
             tc.tile_pool(name="hc", bufs=3) as hpool, \
             tc.tile_pool(name="fold", bufs=2) as fpool, \
             tc.tile_pool(name="cand", bufs=2) as candp, \
             tc.tile_pool(name="fin", bufs=2) as finp, \
             tc.tile_pool(name="gath", bufs=2) as gpool, \
             tc.tile_pool(name="prod", bufs=2) as ppool, \
             tc.tile_pool(name="acc", bufs=1) as accp, \
             tc.tile_pool(name="ps", bufs=2, space="PSUM") as psp:

            # ---- constants ----
            # per-head fold-group member offsets {0,256,...,1792} + h*N_PAD
            ioff_h = []
            for h in range(N_HEADS):
                t = cpool.tile([128, 8], F32, tag=f"ioff{h}")
                nc.gpsimd.iota(t[:], pattern=[[256, 8]], base=h * N_PAD,
                               channel_multiplier=0,
                               allow_small_or_imprecise_dtypes=True)
                ioff_h.append(t)
            # j*256 offset per candidate position (200-array)
            joff = cpool.tile([128, N_SUPERS, 8], F32, tag="joff")
            nc.gpsimd.iota(joff[:], pattern=[[256, N_SUPERS], [0, 8]], base=0,
                           channel_multiplier=0,
                           allow_small_or_imprecise_dtypes=True)
            neg_inf8 = cpool.tile([128, 8], F32, tag="neg_inf8")
            nc.vector.memset(neg_inf8[:], -1e30)

            # ---- query tiles (all m upfront; small) ----
            q_s = []
            aux_s = []
            for m in range(M_TILES):
                qt = qpool.tile([KC, 128], F32R, tag=f"q{m}")
                nc.sync.dma_start(out=qt[:], in_=qpack_d[m])
                q_s.append(qt)
                at = qpool.tile([128, AUXW], F32, tag=f"aux{m}")
                nc.sync.dma_start(out=at[:], in_=aux_d[m])
                aux_s.append(at)

            numneg_all = accp.tile([128, M_TILES], F32, tag="numneg")
            wsum_all = accp.tile([128, M_TILES], F32, tag="wsum")

            for h in range(N_HEADS):
                # per-m-tile candidate stores for both endpoints
                cv = []
                cs = []
                for e in range(2):
                    cvt = candp.tile([128, N_SUPERS, 8], F16, tag=f"cv{e}")
                    cv.append(cvt)
                    cst = candp.tile([128, N_SUPERS, 8], U32, tag=f"cs{e}")
                    cs.append(cst)

                for j in range(N_SUPERS):
                    rhs_s = rpool.tile([KC, SUP], F32R, tag="rhs")
                    nc.sync.dma_start(
                        out=rhs_s[:], in_=rhs_d[h, :, j * SUP:(j + 1) * SUP])
                    for e in range(2):
                        m = 2 * h + e
                        psum = psp.tile([128, SUP], F32, tag="ps")
                        for b in range(SUP // 512):
                            sl = slice(b * 512, (b + 1) * 512)
                            nc.tensor.matmul(psum[:, sl], q_s[m][:], rhs_s[:, sl],
                                             start=True, stop=True)

                        ty = SUPER_TYPE[j]
                        f1 = fpool.tile([128, 1024], F16, tag="f1")
                        f2 = fpool.tile([128, 512], F16, tag="f2")
                        pld = fpool.tile([128, 256], F16, tag="pld")
                        if ty in ("G", "M"):
                            hcv = hpool.tile([128, SUP], F16, tag="hc")
                            nc.scalar.activation(
                                hcv[:], psum[:],
                                mybir.ActivationFunctionType.Copy,
                                bias=0.0, scale=1.0)
                            nc.vector.tensor_tensor(
                                out=f1[:], in0=hcv[:, 0:1024],
                                in1=hcv[:, 1024:2048],
                                op=mybir.AluOpType.max)
                        else:  # 'D': DVE-only (PSUM allows one input per op)
                            c1 = fpool.tile([128, 1024], F16, tag="c1")
                            nc.vector.tensor_copy(c1[:], psum[:, 0:1024])
                            nc.vector.tensor_tensor(
                                out=f1[:], in0=c1[:],
                                in1=psum[:, 1024:2048],
                                op=mybir.AluOpType.max)
                        nc.vector.tensor_tensor(
                            out=f2[:], in0=f1[:, 0:512], in1=f1[:, 512:1024],
                            op=mybir.AluOpType.max)
                        nc.vector.tensor_tensor(
                            out=pld[:], in0=f2[:, 0:256], in1=f2[:, 256:512],
                            op=mybir.AluOpType.max)

                        nc.vector.max(cv[e][:, j], pld[:])
                        nc.vector.max_index(cs[e][:, j], cv[e][:, j], pld[:])
                        if debug and h == 0 and j == 0 and e == 0:
                            pldf = fpool.tile([128, 256], F32, tag="pldf")
                            nc.vector.tensor_copy(pldf[:], pld[:])
                            nc.sync.dma_start(out=dbg_pld_d[:], in_=pldf[:])

                for e in range(2):
                    m = 2 * h + e
                    qn_s = aux_s[m][:, 0:1]
                    qf_s = aux_s[m][:, 1:2]
                    f_row = aux_s[m][:, 2:2 + DIM]
                    q_row = aux_s[m][:, 2 + DIM:2 + 2 * DIM]

                    # ---- pack candidates: qv*8192 + gslot ----
                    cvf = finp.tile([128, N_CAND], F32, tag="cvf")
                    # qv = round(clip((v+42)*32, 0, 2047))
                    nc.scalar.activation(
                        cvf[:], cv[e][:].rearrange("p a b -> p (a b)"),
                        mybir.ActivationFunctionType.Copy,
                        bias=0.0, scale=32.0)
                    nc.vector.tensor_scalar(
                        out=cvf[:], in0=cvf[:], scalar1=1344.0,
                        scalar2=TWO23, op0=mybir.AluOpType.add,
                        op1=mybir.AluOpType.add)
                    nc.vector.tensor_scalar(
                        out=cvf[:], in0=cvf[:], scalar1=TWO23,
                        scalar2=None, op0=mybir.AluOpType.subtract)
                    nc.vector.tensor_scalar(
                        out=cvf[:], in0=cvf[:], scalar1=2047.0,
                        scalar2=0.0, op0=mybir.AluOpType.min,
                        op1=mybir.AluOpType.max)
                    slotf = finp.tile([128, N_CAND], F32, tag="slotf")
                    nc.vector.tensor_copy(
                        slotf[:], cs[e][:].rearrange("p a b -> p (a b)"))
                    gslot = finp.tile([128, N_CAND], F32, tag="gslot")
                    nc.vector.tensor_tensor(
                        out=gslot[:], in0=slotf[:],
                        in1=joff[:].rearrange("p a b -> p (a b)"),
                        op=mybir.AluOpType.add)
                    packed = finp.tile([128, N_CAND], F32, tag="packed")
                    nc.vector.scalar_tensor_tensor(
                        out=packed[:], in0=cvf[:], scalar=8192.0,
                        in1=gslot[:], op0=mybir.AluOpType.mult,
                        op1=mybir.AluOpType.add)
                    if debug and m == 0:
                        cvdbg = finp.tile([128, N_CAND], F32, tag="cvdbg")
                        nc.vector.tensor_copy(
                            cvdbg[:], cv[e][:].rearrange("p a b -> p (a b)"))
                        nc.sync.dma_start(out=dbg_cv_d[:], in_=cvdbg[:])
                        csdbg = finp.tile([128, N_CAND], U32, tag="csdbg")
                        nc.vector.tensor_copy(
                            csdbg[:], cs[e][:].rearrange("p a b -> p (a b)"))
                        nc.sync.dma_start(out=dbg_cs_d[:], in_=csdbg[:])
                        nc.sync.dma_start(out=dbg_pk_d[:], in_=packed[:])

                    # ---- top-16 packed (ties impossible) ----
                    w16 = finp.tile([128, N_WIN], F32, tag="w16")
                    nc.vector.max(w16[:, 0:8], packed[:])
                    prep = finp.tile([128, N_CAND], F32, tag="prep")
                    nc.vector.match_replace(prep[:], w16[:, 0:8], packed[:],
                                            -1e30)
                    nc.vector.max(w16[:, 8:16], prep[:])

                    # ---- decode: gslot16 = w16 mod 8192 -> (j, f) -> base ----
                    qv16 = finp.tile([128, N_WIN], F32, tag="qv16")
                    nc.scalar.activation(qv16[:], w16[:],
                                         mybir.ActivationFunctionType.Copy,
                                         bias=0.0, scale=1.0 / 8192.0)
                    nc.vector.tensor_scalar(
                        out=qv16[:], in0=qv16[:], scalar1=-0.499969482421875,
                        scalar2=TWO23, op0=mybir.AluOpType.add,
                        op1=mybir.AluOpType.add)
                    nc.vector.tensor_scalar(
                        out=qv16[:], in0=qv16[:], scalar1=TWO23,
                        scalar2=None, op0=mybir.AluOpType.subtract)
                    g16 = finp.tile([128, N_WIN], F32, tag="g16")
                    nc.vector.scalar_tensor_tensor(
                        out=g16[:], in0=qv16[:], scalar=-8192.0,
                        in1=w16[:], op0=mybir.AluOpType.mult,
                        op1=mybir.AluOpType.add)
                    # j16 = floor(g16/256); f16 = g16 - 256*j16
                    j16 = finp.tile([128, N_WIN], F32, tag="j16")
                    nc.scalar.activation(j16[:], g16[:],
                                         mybir.ActivationFunctionType.Copy,
                                         bias=0.0, scale=1.0 / 256.0)
                    nc.vector.tensor_scalar(
                        out=j16[:], in0=j16[:], scalar1=-0.498046875,
                        scalar2=TWO23, op0=mybir.AluOpType.add,
                        op1=mybir.AluOpType.add)
                    nc.vector.tensor_scalar(
                        out=j16[:], in0=j16[:], scalar1=TWO23,
                        scalar2=None, op0=mybir.AluOpType.subtract)
                    # base16 = g16 + 1792*j16  (= j*2048 + f)
                    base16 = finp.tile([128, N_WIN], F32, tag="base16")
                    nc.vector.scalar_tensor_tensor(
                        out=base16[:], in0=j16[:], scalar=1792.0,
                        in1=g16[:], op0=mybir.AluOpType.mult,
                        op1=mybir.AluOpType.add)
                    # gid128 = base16 + {0,256,...,1792} + h*N_PAD
                    gidf = finp.tile([128, N_WIN, 8], F32, tag="gidf")
                    nc.vector.tensor_tensor(
                        out=gidf[:],
                        in0=base16[:].rearrange("p (a b) -> p a b", b=1)
                            .to_broadcast((128, N_WIN, 8)),
                        in1=ioff_h[h][:].rearrange("p (a b) -> p a b", a=1)
                            .to_broadcast((128, N_WIN, 8)),
                        op=mybir.AluOpType.add)
                    gidu = finp.tile([128, N_GATH], U32, tag="gidu")
                    nc.vector.tensor_copy(
                        gidu[:], gidf[:].rearrange("p a b -> p (a b)"))
                    if debug and m == 0:
                        nc.sync.dma_start(out=dbg_w16_d[:], in_=w16[:])
                        nc.sync.dma_start(out=dbg_gidu_d[:], in_=gidu[:])

                    # ---- gather the 16 winner groups (272 B rows) ----
                    goff = finp.tile([128, N_WIN], F32, tag="goff")
                    nc.vector.tensor_scalar(
                        out=goff[:], in0=g16[:],
                        scalar1=float(h * N_SUPERS * 256), scalar2=None,
                        op0=mybir.AluOpType.add)
                    goffu = finp.tile([128, N_WIN], U32, tag="goffu")
                    nc.vector.tensor_copy(goffu[:], goff[:])
                    gath = gpool.tile([128, N_WIN, 8, EW], F32, tag="gath")
                    for w in range(N_WIN):
                        nc.gpsimd.indirect_dma_start(
                            out=gath[:, w].rearrange("p k d -> p (k d)"),
                            out_offset=None,
                            in_=embg_d[:],
                            in_offset=IndirectOffsetOnAxis(
                                ap=goffu[:, w:w + 1], axis=0))

                    # ---- exact recompute s = 2 q.e - (qn + en) ----
                    prod = ppool.tile([128, N_WIN, 8, DIM], F32, tag="prod")
                    nc.vector.tensor_tensor(
                        out=prod[:], in0=gath[:, :, :, 0:DIM],
                        in1=q_row.rearrange("p (o u d) -> p o u d", o=1, u=1)
                            .to_broadcast((128, N_WIN, 8, DIM)),
                        op=mybir.AluOpType.mult)
                    dot = finp.tile([128, N_GATH], F32, tag="dot")
                    nc.vector.tensor_reduce(dot[:], prod[:],
                                            axis=mybir.AxisListType.X,
                                            op=mybir.AluOpType.add)
                    t128 = finp.tile([128, N_GATH], F32, tag="t128")
                    nc.vector.tensor_scalar(
                        out=t128[:],
                        in0=gath[:, :, :, DIM:DIM + 1]
                            .rearrange("p w k o -> p (w k o)"),
                        scalar1=qn_s, scalar2=None,
                        op0=mybir.AluOpType.add)
                    s128 = finp.tile([128, N_GATH], F32, tag="s128")
                    nc.vector.scalar_tensor_tensor(
                        out=s128[:], in0=dot[:], scalar=2.0, in1=t128[:],
                        op0=mybir.AluOpType.mult,
                        op1=mybir.AluOpType.subtract)
                    if debug and m == 0:
                        nc.sync.dma_start(out=dbg_s128_d[:], in_=s128[:])

                    # ---- exact top-9, drop rank-1 (self) ----
                    m1 = finp.tile([128, 1], F32, tag="m1")
                    nc.vector.tensor_reduce(m1[:], s128[:],
                                            axis=mybir.AxisListType.X,
                                            op=mybir.AluOpType.max)
                    m1x8 = finp.tile([128, 8], F32, tag="m1x8")
                    nc.vector.tensor_copy(m1x8[:], neg_inf8[:])
                    nc.vector.tensor_copy(m1x8[:, 0:1], m1[:])
                    srep = finp.tile([128, N_GATH], F32, tag="srep")
                    nc.vector.match_replace(srep[:], m1x8[:], s128[:], -1e30)
                    w8 = finp.tile([128, 8], F32, tag="w8")
                    nc.vector.max(w8[:], srep[:])
                    srep2 = finp.tile([128, N_GATH], F32, tag="srep2")
                    nc.vector.match_replace(srep2[:], w8[:], srep[:], 1e30)
                    mask = finp.tile([128, N_GATH], F32, tag="mask")
                    nc.vector.tensor_scalar(out=mask[:], in0=srep2[:],
                                            scalar1=1e29, scalar2=None,
                                            op0=mybir.AluOpType.is_ge)
                    gidsel = finp.tile([128, N_GATH], F32, tag="gidsel")
                    nc.vector.scalar_tensor_tensor(
                        out=gidsel[:], in0=gidf[:].rearrange("p a b -> p (a b)"),
                        scalar=1.0, in1=mask[:],
                        op0=mybir.AluOpType.add, op1=mybir.AluOpType.mult)
                    wgidf = finp.tile([128, 8], F32, tag="wgidf")
                    nc.vector.max(wgidf[:], gidsel[:])
                    wgidu = finp.tile([128, 8], U32, tag="wgidu")
                    nc.vector.tensor_scalar(
                        out=wgidu[:], in0=wgidf[:], scalar1=-1.0,
                        scalar2=None, op0=mybir.AluOpType.add)
                    nc.sync.dma_start(out=dbg_gid_d[m], in_=wgidu[:])
                    nc.sync.dma_start(out=dbg_s_d[m], in_=w8[:])

                    # ---- gather the 8 winners, field dots, weights ----
                    g2 = gpool.tile([128, 8, EW], F32, tag="g2")
                    for k in range(8):
                        nc.gpsimd.indirect_dma_start(
                            out=g2[:, k], out_offset=None,
                            in_=embn_d[:],
                            in_offset=IndirectOffsetOnAxis(
                                ap=wgidu[:, k:k + 1], axis=0))
                    prod8 = finp.tile([128, 8, DIM], F32, tag="prod8")
                    nc.vector.tensor_tensor(
                        out=prod8[:], in0=g2[:, :, 0:DIM],
                        in1=q_row.rearrange("p (o d) -> p o d", o=1)
                            .to_broadcast((128, 8, DIM)),
                        op=mybir.AluOpType.mult)
                    dot8 = finp.tile([128, 8], F32, tag="dot8")
                    nc.vector.tensor_reduce(dot8[:], prod8[:],
                                            axis=mybir.AxisListType.X,
                                            op=mybir.AluOpType.add)
                    t8 = finp.tile([128, 8], F32, tag="t8")
                    nc.vector.tensor_scalar(out=t8[:], in0=g2[:, :, DIM],
                                            scalar1=qn_s, scalar2=None,
                                            op0=mybir.AluOpType.add)
                    s8 = finp.tile([128, 8], F32, tag="s8")
                    nc.vector.scalar_tensor_tensor(
                        out=s8[:], in0=dot8[:], scalar=2.0, in1=t8[:],
                        op0=mybir.AluOpType.mult,
                        op1=mybir.AluOpType.subtract)
                    nc.vector.tensor_scalar(out=s8[:], in0=s8[:], scalar1=0.0,
                                            scalar2=None,
                                            op0=mybir.AluOpType.min)
                    dist8 = finp.tile([128, 8], F32, tag="dist8")
                    nc.scalar.activation(dist8[:], s8[:],
                                         mybir.ActivationFunctionType.Sqrt,
                                         bias=0.0, scale=-1.0)
                    wexp8 = finp.tile([128, 8], F32, tag="wexp8")
                    nc.scalar.activation(wexp8[:], dist8[:],
                                         mybir.ActivationFunctionType.Exp,
                                         bias=1.0, scale=-1.0)
                    prodf8 = finp.tile([128, 8, DIM], F32, tag="prodf8")
                    nc.vector.tensor_tensor(
                        out=prodf8[:], in0=g2[:, :, 0:DIM],
                        in1=f_row.rearrange("p (o d) -> p o d", o=1)
                            .to_broadcast((128, 8, DIM)),
                        op=mybir.AluOpType.mult)
                    u8 = finp.tile([128, 8], F32, tag="u8")
                    nc.vector.tensor_reduce(u8[:], prodf8[:],
                                            axis=mybir.AxisListType.X,
                                            op=mybir.AluOpType.add)
                    scrap8 = finp.tile([128, 8], F32, tag="scrap8")
                    nc.vector.scalar_tensor_tensor(
                        out=scrap8[:], in0=u8[:], scalar=qf_s, in1=wexp8[:],
                        op0=mybir.AluOpType.subtract,
                        op1=mybir.AluOpType.mult,
                        accum_out=numneg_all[:, m:m + 1])
                    nc.vector.tensor_reduce(wsum_all[:, m:m + 1], wexp8[:],
                                            axis=mybir.AxisListType.X,
                                            op=mybir.AluOpType.add)

            # ---- combine heads: pred = sigmoid(mean_h num_h / den_h) ----
            sp = finp
            nsum2 = sp.tile([128, N_HEADS], F32, tag="nsum2")
            nc.vector.tensor_reduce(
                nsum2[:], numneg_all[:].rearrange("p (h e) -> p h e", e=2),
                axis=mybir.AxisListType.X, op=mybir.AluOpType.add)
            den = sp.tile([128, N_HEADS], F32, tag="den")
            nc.vector.tensor_reduce(
                den[:], wsum_all[:].rearrange("p (h e) -> p h e", e=2),
                axis=mybir.AxisListType.X, op=mybir.AluOpType.add)
            den8 = sp.tile([128, N_HEADS], F32, tag="den8")
            nc.vector.tensor_scalar(out=den8[:], in0=den[:],
                                    scalar1=float(N_SENT), scalar2=None,
                                    op0=mybir.AluOpType.add)
            rden = sp.tile([128, N_HEADS], F32, tag="rden")
            nc.vector.reciprocal(rden[:], den8[:])
            ratio = sp.tile([128, N_HEADS], F32, tag="ratio")
            nc.vector.tensor_tensor(out=ratio[:], in0=nsum2[:], in1=rden[:],
                                    op=mybir.AluOpType.mult)
            ssum = sp.tile([128, 1], F32, tag="ssum")
            nc.vector.tensor_reduce(ssum[:], ratio[:],
                                    axis=mybir.AxisListType.X,
                                    op=mybir.AluOpType.add)
            preds_s = sp.tile([128, 1], F32, tag="preds")
            nc.scalar.activation(preds_s[:], ssum[:],
                                 mybir.ActivationFunctionType.Sigmoid,
                                 bias=0.0, scale=-1.0 / N_HEADS)
            nc.sync.dma_start(out=preds_d[:], in_=preds_s[:])

    return nc


def _prep_inputs(embeds, field, edges):
    """Host-side layout prep + per-core sharding."""
    embeds = np.asarray(embeds, dtype=np.float32)
    field = np.asarray(field, dtype=np.float32)
    edges = np.asarray(edges)

    en = np.sum(np.square(embeds), axis=-1, dtype=np.float32)
    rhs_aug = np.empty((N_HEADS, KC, N_PAD), dtype=np.float32)
    rhs_aug[:, :DIM, :N_NODES] = embeds.transpose(0, 2, 1)
    rhs_aug[:, DIM, :N_NODES] = en
    rhs_aug[:, :DIM, N_NODES:] = 0.0
    rhs_aug[:, DIM, N_NODES:] = 60000.0  # pad columns: s = -60000 (fp16-safe)

    embn = np.zeros((N_HEADS * N_PAD, EW), dtype=np.float32)
    embn3 = embn.reshape(N_HEADS, N_PAD, EW)
    embn3[:, :N_NODES, :DIM] = embeds
    embn3[:, :N_NODES, DIM] = en
    embn3[:, N_NODES:, DIM] = 60000.0

    # group-major table: row (h, j, f) = members {j*2048 + f + 256k}
    embg = np.ascontiguousarray(
        embn3.reshape(N_HEADS, N_SUPERS, 8, 256, EW)
             .transpose(0, 1, 3, 2, 4)
             .reshape(N_HEADS * N_SUPERS * 256, 8 * EW))

    in_maps = []
    for c in range(N_CORES):
        sl = slice(c * EDGES_PER_CORE, (c + 1) * EDGES_PER_CORE)
        qpack = np.zeros((M_TILES, KC, 128), dtype=np.float32)
        aux = np.zeros((M_TILES, 128, AUXW), dtype=np.float32)
        for m in range(M_TILES):
            h, e = m // 2, m % 2
            nodes = edges[e, sl]
            q = embeds[h, nodes]                      # (128, 32)
            f = field[h, nodes]                       # (128, 32)
            qpack[m, :DIM] = (2.0 * q).T
            qpack[m, DIM] = -1.0
            aux[m, :, 0] = np.einsum('bd,bd->b', q, q)
            aux[m, :, 1] = np.einsum('bd,bd->b', q, f)
            aux[m, :, 2:2 + DIM] = f
            aux[m, :, 2 + DIM:] = q
        in_maps.append({
            "rhs_aug": rhs_aug, "embn": embn, "embg": embg,
            "qpack": qpack, "aux": aux,
        })
    return in_maps


def kernel(embeds, field, edges):
    from concourse.bass_utils import run_bass_kernel_spmd

    nc = _build_program()
    nc.finalize()
    in_maps = _prep_inputs(embeds, field, edges)
    core_ids = list(range(N_CORES))
    trace = bool(os.environ.get("KNN_TRACE"))
    tmpdir = os.environ.get("KNN_TRACE_DIR") or None
    out = run_bass_kernel_spmd(nc, in_maps, core_ids, trace=trace,
                               tmpdir=tmpdir)
    LAST["results"] = out
    preds = np.concatenate(
        [out.results[c]["preds"][:, 0] for c in range(N_CORES)])
    return preds.astype(np.float32)
